# revision 1
# baseline (speedup 1.0000x reference)
"""Trainium2 Bass kernel for nn_DifferentiableForwardModel.

Model: out[b,k] = PSF_conv( sum_lam bilinear_shift(pad(cube[b,lam]); dy[k,lam], dx[k,lam]) )
Shapes (hardcoded): cube (4,96,256,256) f32, dx/dy (4,96) f32, psf (15,15) f32
Output: (4,4,288,288) f32.

Sharding: 8 cores = (k in 0..4) x (b-half in 0..2); each core computes the two
full output images (k, 2*bh+{0,1}) — no cross-core reduction needed.

Per-core pipeline (all shift-dependent quantities enter as DATA so all 8 cores
run one identical SPMD program):
  host:  integer x-shift baked into the upload layout (fp16), per (k,lam)
  DVE:   fused scalar_tensor_tensor does the fractional x-blend
         imX = imS[.,u]*s + imS[.,u+1], s=(1-tx)/tx (tx folded into Wy)
  PE:    banded weight matmuls do the fractional+integer y-shift AND the
         lambda-sum via PSUM accumulation (weights = uploaded data)
  PE:    15x15 PSF conv as banded weight matmuls (contract over rows,
         column taps via free-dim offsets), PSUM accumulate
"""
import numpy as np

import sys
import types

# This container's thin axon client has no antenv.axon_hooks; shim it so
# run_bass_kernel_spmd's trace path degrades gracefully instead of raising.
try:
    from antenv import axon_hooks as _ah  # noqa: F401
except ImportError:
    _m = types.ModuleType("antenv.axon_hooks")
    _m.get_axon_ntff_profile_hook = lambda: None
    sys.modules["antenv.axon_hooks"] = _m

import concourse.bass as bass
import concourse.bacc as bacc
import concourse.mybir as mybir
import concourse.tile as tile
from concourse import bass_utils
from concourse.bass_interp import get_hw_module

# problem shapes
B, NL, H, W = 4, 96, 256, 256
PAD = 16
HO = WO = 288
KS, KH = 15, 7
N_CORES = 8

WS = 296        # uploaded slot frame width (x), with margins
N_PE = 1        # trailing slots whose x-blend runs on PE (2-tap matmuls)
EPS_T = 1e-3    # clamp for fractional x part (keeps fold scalar bounded)
F16 = mybir.dt.float16
F32 = mybir.dt.float32

_cached = {}


def _build_program(replicas=1):
    """Build the SPMD Bass program (same for every core; all shift data via inputs).

    replicas > 1 emits the whole compute body multiple times (same inputs,
    same outputs) for steady-state HW timing measurements.
    """
    nc = bacc.Bacc("TRN2", target_bir_lowering=False, debug=False,
                   num_devices=N_CORES)

    ims_d = nc.dram_tensor("ims", [NL, 128, 2, 2, WS], F16, kind="ExternalInput").ap()
    wy_d = nc.dram_tensor("wy", [NL, 128, 160], F16, kind="ExternalInput").ap()
    wy4_d = nc.dram_tensor("wy4", [N_PE, 128, 2, 160], F16, kind="ExternalInput").ap()
    scal_d = nc.dram_tensor("scal", [128, NL], F32, kind="ExternalInput").ap()
    cw_d = nc.dram_tensor("cw", [112, 3, KS, 96], F16, kind="ExternalInput").ap()
    out_d = nc.dram_tensor("out", [2, HO, WO], F32, kind="ExternalOutput").ap()

    mult = mybir.AluOpType.mult
    add = mybir.AluOpType.add

    with tile.TileContext(nc) as tc:
      for _rep in range(replicas):
        with (
            tc.tile_pool(name="const", bufs=1) as constp,
            tc.tile_pool(name="ims", bufs=10) as imsp,
            tc.tile_pool(name="imx", bufs=6) as imxp,
            tc.tile_pool(name="af", bufs=1) as afp,
            tc.tile_pool(name="ac", bufs=1) as acp,
            tc.tile_pool(name="oc", bufs=2) as ocp,
        ):
            # wy first, in chunks, so PE's slot-0 matmul unblocks early;
            # cw (conv weights) last — only needed at the tail.
            scal_t = constp.tile([128, NL], F32, tag="scal")
            nc.scalar.dma_start(scal_t[:], scal_d)
            wy_t = constp.tile([128, NL * 160], F16, tag="wyall")
            WYC = 12
            for g in range(0, NL, WYC):
                nc.scalar.dma_start(
                    wy_t[:, g * 160:(g + WYC) * 160].rearrange(
                        "p (j m) -> p j m", j=WYC),
                    wy_d[g:g + WYC].rearrange("j p m -> p j m"))
            wy4_t = constp.tile([128, N_PE * 2 * 160], F16, tag="wy4all")
            nc.scalar.dma_start(
                wy4_t[:].rearrange("p (j t m) -> p j t m", j=N_PE, t=2),
                wy4_d.rearrange("j p t m -> p j t m"))
            cw_t = constp.tile([112, 3 * KS * 96], F16, tag="cw")
            nc.scalar.dma_start(cw_t[:], cw_d.rearrange("p m k q -> p (m k q)"))

            with tc.tile_pool(name="ps", bufs=1, space="PSUM") as psp:
                ps = [[psp.tile([128, WO], F32, tag=f"ps{b}{m}", name=f"ps{b}{m}") for m in range(3)]
                      for b in range(2)]

                # ---- stage 1: x-blend (DVE) + y-shift/lambda-sum (PE) ----
                for j in range(NL):
                    ims_t = imsp.tile([128, 2 * 2 * WS], F16, tag="ims")
                    nc.sync.dma_start(
                        ims_t[:], ims_d[j].rearrange("p r b u -> p (r b u)"))
                    imsv = ims_t[:].rearrange("p (r b u) -> p r b u", r=2, b=2)

                    if j < NL - N_PE:
                        # DVE path: fused fractional x-blend, then 4 matmuls
                        imx_t = imxp.tile([128, 2 * 2 * WO], F16, tag="imx")
                        imxv = imx_t[:].rearrange("p (r b x) -> p r b x", r=2, b=2)
                        nc.vector.scalar_tensor_tensor(
                            imxv, imsv[:, :, :, 4:4 + WO], scal_t[:, j:j + 1],
                            imsv[:, :, :, 5:5 + WO], op0=mult, op1=add)
                        for h in range(2):      # weight block (stationary reuse)
                            lhs = (wy_t[:, j * 160:j * 160 + 128] if h == 0
                                   else wy_t[:, j * 160 + 128:j * 160 + 160])
                            for i in range(2):  # input row chunk (K)
                                for b in range(2):
                                    nc.tensor.matmul(
                                        ps[b][i + h][0:128, :] if h == 0
                                        else ps[b][i + h][0:32, :],
                                        lhs,
                                        imxv[:, i, b, :],
                                        start=(j == 0 and not (h == 1 and i == 0)),
                                        stop=False,
                                    )
                    else:
                        # PE path: both x-taps as separate tap-scaled matmuls
                        j2 = j - (NL - N_PE)
                        for h in range(2):
                            for i in range(2):
                                for b in range(2):
                                    for t in range(2):
                                        base = (j2 * 2 + t) * 160
                                        lhs = (wy4_t[:, base:base + 128] if h == 0
                                               else wy4_t[:, base + 128:base + 160])
                                        nc.tensor.matmul(
                                            ps[b][i + h][0:128, :] if h == 0
                                            else ps[b][i + h][0:32, :],
                                            lhs,
                                            imsv[:, i, b, 4 + t:4 + t + WO],
                                            start=False,
                                            stop=(j == NL - 1 and t == 1
                                                  and not (h == 0 and i == 1)),
                                        )

                # ---- evacuate stage-1 psum to fp16 acc (flat, then conv layout) ----
                af = [[afp.tile([128, WO], F16, tag=f"af{b}{m}", name=f"af{b}{m}") for m in range(3)]
                      for b in range(2)]
                for b in range(2):
                    for m in range(3):
                        nc.scalar.copy(af[b][m][:], ps[b][m][:])

            ac = [[acp.tile([112, 304], F16, tag=f"ac{b}{m}", name=f"ac{b}{m}") for m in range(3)]
                  for b in range(2)]
            for b in range(2):
                for m in range(3):
                    nc.gpsimd.memset(ac[b][m][:], 0.0)
                # rows of ac[m] = acc rows [96m-8, 96m+104) ; cols 8:296 = xo
                nc.sync.dma_start(ac[b][0][8:112, 8:8 + WO], af[b][0][0:104, :])
                nc.sync.dma_start(ac[b][1][0:40, 8:8 + WO], af[b][0][88:128, :])
                nc.sync.dma_start(ac[b][1][40:112, 8:8 + WO], af[b][1][0:72, :])
                nc.sync.dma_start(ac[b][2][0:72, 8:8 + WO], af[b][1][56:128, :])
                nc.sync.dma_start(ac[b][2][72:104, 8:8 + WO], af[b][2][0:32, :])

            # ---- PSF conv (PE, banded row-contraction; col taps via offsets) ----
            with tc.tile_pool(name="pc", bufs=1, space="PSUM") as pcp:
                for b in range(2):
                    for m in range(3):
                        pc_t = pcp.tile([96, WO], F32, tag=f"pc{b % 2}{m}", name=f"pc{b}{m}")
                        for kx in range(KS):
                            nc.tensor.matmul(
                                pc_t[:],
                                cw_t[:, (m * KS + kx) * 96:(m * KS + kx + 1) * 96],
                                ac[b][m][:, 1 + kx:1 + kx + WO],
                                start=(kx == 0), stop=(kx == KS - 1),
                            )
                        oc_t = ocp.tile([96, WO], F32, tag="oc")
                        nc.scalar.copy(oc_t[:], pc_t[:])
                        nc.scalar.dma_start(out_d[b, 96 * m:96 * (m + 1), :], oc_t[:])

    nc.compile()
    nc.m = get_hw_module(nc.m)
    return nc


def _decompose(d):
    c = np.ceil(d)
    return c.astype(np.int64), (c - d)


def _build_inputs(cube, dx, dy, psf_kernel):
    """Per-core input arrays. Core c handles k=c//2, b in {2*(c%2), 2*(c%2)+1}."""
    cube16 = cube.astype(np.float16)
    cxs, txs = _decompose(np.asarray(dx, np.float64))
    cys, tys = _decompose(np.asarray(dy, np.float64))
    txs0 = txs.copy()
    txs = np.clip(txs, EPS_T, 1.0)
    assert cxs.min() >= -20 and cxs.max() <= 20, "x shift out of supported range"
    assert cys.min() >= -14 and cys.max() <= 14, "y shift out of supported range"

    # conv weights (same for all cores): cw[p, m, kx, q] = psf[yi-yo+7, kx]
    # with yi = 96m-8+p, yo = 96m+q
    cw = np.zeros((112, 3, KS, 96), np.float16)
    p_idx = np.arange(112)[:, None]
    q_idx = np.arange(96)[None, :]
    for m in range(3):
        yi = 96 * m - 8 + p_idx
        ky = (yi - (96 * m + q_idx)) + KH          # [112, 96]
        valid = (ky >= 0) & (ky < KS) & (yi >= 0) & (yi < HO)
        for kx in range(KS):
            blk = np.zeros((112, 96), np.float32)
            blk[valid] = np.asarray(psf_kernel, np.float32)[ky[valid], kx]
            cw[:, m, kx, :] = blk.astype(np.float16)

    in_maps = []
    for c in range(N_CORES):
        k, bh = c // 2, c % 2
        bs = [2 * bh, 2 * bh + 1]

        ims = np.zeros((NL, 128, 2, 2, WS), np.float16)
        wy = np.zeros((NL, 128, 160), np.float16)
        wy4 = np.zeros((N_PE, 128, 2, 160), np.float16)
        scal = np.zeros((128, NL), np.float32)
        for j in range(NL):
            cx, tx = int(cxs[k, j]), float(txs[k, j])
            cy, ty = int(cys[k, j]), float(tys[k, j])
            tx0 = float(txs0[k, j])
            lo = 20 + cx
            # tmp[bl, v, u]
            tmp = np.zeros((2, 256, WS), np.float16)
            tmp[:, :, lo:lo + 256] = cube16[bs, j]
            # -> [p, r, bl, u] with v = 128r + p
            ims[j] = tmp.reshape(2, 2, 128, WS).transpose(2, 1, 0, 3)
            scal[:, j] = (1.0 - tx) / tx
            # wy blocks: identical for both K-chunks; wy[p, h, m]:
            #   m = p + (cy + 16 - a - 128h), value tx * wy_a
            for a, wa in ((0, 1.0 - ty), (1, ty)):
                for hh, base, mw in ((0, 0, 128), (1, 128, 32)):
                    dgl = cy + 16 - a - 128 * hh
                    p0, p1 = max(0, -dgl), min(128, mw - dgl)
                    if p0 < p1:
                        pr = np.arange(p0, p1)
                        wy[j, pr, base + pr + dgl] = np.float16(tx * wa)
                        if j >= NL - N_PE:
                            txr = float(cxs[k, j] - dx[k, j]) if False else None
                            for t, wx in ((0, 1.0 - tx0), (1, tx0)):
                                wy4[j - (NL - N_PE), pr, t, base + pr + dgl] = \
                                    np.float16(wx * wa)
        in_maps.append({"ims": ims, "wy": wy, "wy4": wy4, "scal": scal,
                        "cw": cw})
    return in_maps


def _run(cube, dx, dy, psf_kernel, trace=False):
    if "nc" not in _cached:
        _cached["nc"] = _build_program()
    nc = _cached["nc"]
    in_maps = _build_inputs(np.asarray(cube, np.float32), np.asarray(dx),
                            np.asarray(dy), np.asarray(psf_kernel))
    res = bass_utils.run_bass_kernel_spmd(
        nc, in_maps, core_ids=list(range(N_CORES)), trace=trace)
    out = np.zeros((B, 4, HO, WO), np.float32)
    for c in range(N_CORES):
        k, bh = c // 2, c % 2
        o = res.results[c]["out"]
        out[2 * bh, k] = o[0]
        out[2 * bh + 1, k] = o[1]
    return out, res


def kernel(cube, dx, dy, psf_kernel):
    out, _ = _run(cube, dx, dy, psf_kernel, trace=False)
    return out

